# revision 5
# baseline (speedup 1.0000x reference)
"""MixerAttention (GQA + QK-RMSNorm + RoPE + causal) Trainium2 kernel, v2.

Sharding: 8 cores = batch(2) x kv-head(4). Fully local per core — no collectives.

v2 redesign (v1 was DVE-bound at ~94% busy, ACT 72%, PE 67%):
  - Projections run in bf16 (fp8 was measured at 3.4e-2 max-normalized error
    vs the 2e-2 gate -- peaked softmax amplifies score noise -- while the
    all-bf16/fp16 pipeline lands at 4e-3).
  - V is projected directly in natural (token, DH) orientation by swapping
    stationary/moving operands, eliminating the PE transposes and their
    staging copies.
  - All DVE chain math is 16-bit (bf16/fp16) in SBUF, hitting the DVE 2x/4x
    packed modes; only PSUM-touching ops stay 4-byte.
  - Softmax runs shifted: es = exp(s - 9) <= e^2.5 so es/acc live in fp16;
    the shift cancels in the normalization. Causal masking is a fp16
    multiply on es (mask01), not a -inf add on PSUM scores.
  - Score blocks are exp'd in fused pairs ([128, 2, 512] PSUM tiles) to
    halve ACT instruction count; denominators accumulate on the DVE in fp16
    and finish with one fp16 ones-matmul per (head, q-chunk).
  - Projections for window n+1 are emitted before attention(i=n) so the PE
    has dense work while attention waits on ACT exps.
"""
import sys

sys.path.insert(0, "/opt/trn_rl_repo")
from contextlib import ExitStack

import ml_dtypes
import numpy as np
import concourse.bacc as bacc
import concourse.mybir as mybir
import concourse.tile as tile
from concourse.bass_utils import run_bass_kernel_spmd

F32 = mybir.dt.float32
BF16 = mybir.dt.bfloat16
FP16 = mybir.dt.float16
FP8 = mybir.dt.float8e4
AF = mybir.ActivationFunctionType
DR = mybir.MatmulPerfMode.DoubleRow

B, T, D = 2, 2048, 2048
H, HKV, DH = 16, 4, 128
G = H // HKV                    # q heads per kv head (per core)
EPS = 1.1920928955078125e-07
ROPE_BASE = 10000.0
NCORES = 8

CSH = 9.0                       # softmax shift: es = exp(s - CSH), s in [-11.4, 11.4]

P = 128                         # partitions
DCH = D // P                    # 16 contraction chunks
NT = 4                          # column windows of 512
TC = T // NT                    # 512
EQ = G * DH                     # 512
ETOT = EQ + DH + DH             # 768
QC = 512                        # attention q-chunk == TC
KC = 128                        # attention k-chunk
NKC = T // KC                   # 16
MK, MV = G, G + 1               # m-tile indices of k and v rows
BQ = float(P) * EPS             # rms eps bias, q (1/sqrt(DH) folded via scale=1)
BK = EPS                        # rms eps bias, k


def _proj_wave(nc, pools, wt, xns, n, ms):
    """One projection wave for window n: bf16-accumulate the m-tiles in
    `ms` ([P,TC] out each). Returns dict m -> psum tile."""
    cps = pools
    psl = {m: cps.tile([P, TC], F32, tag="pj", bufs=2, name=f"pj_{n}_{m}") for m in ms}
    for d in range(DCH):
        xn = xns[(n, d)]
        for m in ms:
            nc.tensor.matmul(
                psl[m],
                wt[:, d, m * P : (m + 1) * P],
                xn,
                start=d == 0,
                stop=d == DCH - 1,
            )
    return psl


def _proj_vnat(nc, cps, wt, xns, n, tb):
    """Natural-orientation V projection for token block tb of window n:
    out [P tokens, KC dh] — one sequential accumulation group per psum bank
    (PSUM allows a single live group per 2KB zero region)."""
    vn = cps.tile([P, TC], F32, tag="pj", bufs=2, name=f"vn_{n}_{tb}")
    for d in range(DCH):
        xn = xns[(n, d)]
        nc.tensor.matmul(
            vn[:, 0:KC],
            xn[:, tb * KC : (tb + 1) * KC],
            wt[:, d, MV * P : (MV + 1) * P],
            start=d == 0,
            stop=d == DCH - 1,
        )
    return vn


def _rope(nc, sp, srcn, dst, dst0, ropeC, ropeS, n, label):
    """dst[:, dst0:dst0+TC] = srcn*C + rot_half(srcn)*S, all bf16 on DVE.
    The rotate-half uses partition-crossed copies (walrus allows those but
    not partition-crossed tensor_tensor inputs); ropeS holds [s; -s]."""
    c0 = n * TC
    h = P // 2
    tmp = sp.tile([P, TC], BF16, tag="rtmp", name=f"rtmp_{label}")
    nc.vector.tensor_copy(tmp[0:h, :], srcn[h:P, :])
    nc.vector.tensor_copy(tmp[h:P, :], srcn[0:h, :])
    t1 = sp.tile([P, TC], BF16, tag="rt1", name=f"rt1_{label}")
    nc.vector.tensor_mul(t1, srcn, ropeC[:, c0 : c0 + TC])
    nc.vector.tensor_mul(tmp, tmp, ropeS[:, c0 : c0 + TC])
    nc.vector.tensor_add(dst[:, dst0 : dst0 + TC], t1, tmp)


def _body(nc, tc, ctx):
    XT = nc.cur_io["xT"]
    WT = nc.cur_io["wT"]
    RC = nc.cur_io["ropeC"]
    RS_ = nc.cur_io["ropeS"]
    MSK = nc.cur_io["mask2"]
    YT = nc.cur_io["yT"]

    constp = ctx.enter_context(tc.tile_pool(name="const", bufs=1))
    finp = ctx.enter_context(tc.tile_pool(name="final", bufs=1))
    wp = ctx.enter_context(tc.tile_pool(name="wp", bufs=1))
    xp = ctx.enter_context(tc.tile_pool(name="xp", bufs=17))
    stg = ctx.enter_context(tc.tile_pool(name="stg", bufs=10))
    sp = ctx.enter_context(tc.tile_pool(name="sp", bufs=3))
    qsc = ctx.enter_context(tc.tile_pool(name="qsc", bufs=10))
    asb = ctx.enter_context(tc.tile_pool(name="asb", bufs=6))
    asb2 = ctx.enter_context(tc.tile_pool(name="asb2", bufs=2))
    cps = ctx.enter_context(tc.tile_pool(name="cps", bufs=1, space="PSUM"))

    # weights split per d-chunk pair and interleaved with window-0 x loads so
    # the first projection matmuls unblock fast; x in [P, TC] bf16 tiles
    wt = wp.tile([P, DCH, ETOT], BF16, tag="wt")
    xns = {}

    def load_x(n, d):
        xn = xp.tile([P, TC], BF16, tag="xn", name=f"xn_{n}_{d}")
        nc.sync.dma_start(out=xn, in_=XT[d, :, n, :])
        xns[(n, d)] = xn

    def load_x23(d):
        xn = xp.tile([P, 2, TC], BF16, tag="xn2", bufs=17, name=f"xn23_{d}")
        nc.sync.dma_start(out=xn, in_=XT[d, :, 2:4, :])
        xns[(2, d)] = xn[:, 0, :]
        xns[(3, d)] = xn[:, 1, :]

    for d2 in range(DCH // 2):
        nc.sync.dma_start(
            out=wt[:, 2 * d2 : 2 * d2 + 2, :], in_=WT[:, 2 * d2 : 2 * d2 + 2, :]
        )
        load_x(0, 2 * d2)
        load_x(0, 2 * d2 + 1)

    mask2 = constp.tile([P, 2 * KC], FP16, tag="mask2")
    nc.sync.dma_start(out=mask2, in_=MSK[:, :])
    tri01 = mask2[:, KC : 2 * KC]
    # rope halves are duplicated ([c;c], [s;-s]) — upload 64 rows, fill the
    # second partition half on-chip
    ropeC = constp.tile([P, T], BF16, tag="ropeC")
    nc.sync.dma_start(out=ropeC[0 : P // 2, :], in_=RC[:, :])
    nc.vector.tensor_copy(ropeC[P // 2 : P, :], ropeC[0 : P // 2, :])
    ropeS = constp.tile([P, T], BF16, tag="ropeS")
    nc.sync.dma_start(out=ropeS[0 : P // 2, :], in_=RS_[:, :])
    nc.vector.tensor_scalar_mul(ropeS[P // 2 : P, :], ropeS[0 : P // 2, :], -1.0)
    ones_bf = constp.tile([P, P], BF16, tag="ones_bf")
    nc.vector.memset(ones_bf, 1.0)
    ones_h = constp.tile([P, P], FP16, tag="ones_h")
    nc.vector.memset(ones_h, 1.0)
    bq = constp.tile([P, 1], F32, tag="bq")
    nc.vector.memset(bq, BQ)
    bk = constp.tile([P, 1], F32, tag="bk")
    nc.vector.memset(bk, BK)
    mC = constp.tile([P, 1], F32, tag="mC")
    nc.vector.memset(mC, -CSH)

    KTr = finp.tile([P, T], BF16, tag="KTr")
    Vnat = finp.tile([P, NKC, KC], FP16, tag="Vnat")

    def stage_of(psl, n, m):
        st = stg.tile([P, TC], BF16, tag="stage", name=f"st_{n}_{m}")
        nc.vector.tensor_copy(st, psl[m])
        return st

    def window_stream(n, qtrs):
        """Generator emitting projections + chains for window n in small
        batches (yield points). Drained as PE-filler inside attention(n-1),
        so the in-order PE queue interleaves ready projection matmuls with
        exp-gated attention matmuls. Also prefetches x for window n+1.
        Stores the window's qtr dict into qtrs[n]."""
        if n == 1:
            for d in range(0, DCH, 4):
                for dd in range(d, d + 4):
                    load_x23(dd)
                yield
        elif n == 0:
            for d in range(0, DCH, 4):
                for dd in range(d, d + 4):
                    load_x(1, dd)
                yield

        def mwave(ms):
            psl = {
                m: cps.tile([P, TC], F32, tag="pj", bufs=2, name=f"pj_{n}_{m}")
                for m in ms
            }
            for d in range(DCH):
                for m in ms:
                    nc.tensor.matmul(
                        psl[m],
                        wt[:, d, m * P : (m + 1) * P],
                        xns[(n, d)],
                        start=d == 0,
                        stop=d == DCH - 1,
                    )
                if len(ms) * (d + 1) % 4 == 0:
                    yield
            for m in ms:
                stage[m] = stage_of(psl, n, m)
            yield

        def kchain():
            sqk = sp.tile([P, TC], BF16, tag="sq", name=f"sqk_{n}")
            nc.vector.tensor_mul(sqk, stage[MK], stage[MK])
            ssbk = cps.tile([P, 2, QC], F32, tag="sps", bufs=2, name=f"ssbk_{n}")
            nc.tensor.matmul(ssbk[:, 0, :], ones_bf, sqk, start=True, stop=True)
            yield
            lntk = sp.tile([P, TC], F32, tag="lnt", name=f"lntk_{n}")
            nc.scalar.activation(
                lntk, ssbk[:, 0, :], AF.Ln, scale=1.0 / P, bias=bk[:, :]
            )
            rsk = sp.tile([P, TC], FP16, tag="rs", name=f"rsk_{n}")
            nc.scalar.activation(rsk, lntk, AF.Exp, scale=-0.5)
            yield
            srcnk = sp.tile([P, TC], BF16, tag="srcn", name=f"srcnk_{n}")
            nc.vector.tensor_mul(srcnk, stage[MK], rsk)
            yield
            _rope(nc, sp, srcnk, KTr, n * TC, ropeC, ropeS, n, f"k{n}")
            yield

        def qchains(pair):
            sqs = {}
            ssb = cps.tile(
                [P, 2, QC], F32, tag="sps", bufs=2, name=f"ssb_{n}_{pair[0]}"
            )
            for u, g in enumerate(pair):
                sqs[g] = sp.tile([P, TC], BF16, tag="sq", name=f"sq_{n}_{g}")
                nc.vector.tensor_mul(sqs[g], stage[g], stage[g])
                nc.tensor.matmul(ssb[:, u, :], ones_bf, sqs[g], start=True, stop=True)
                yield
            lnt = sp.tile([P, 2, TC], F32, tag="lnt2", name=f"lnt_{n}_{pair[0]}")
            nc.scalar.activation(lnt, ssb, AF.Ln, scale=1.0, bias=bq[:, :])
            rs = sp.tile([P, 2, TC], FP16, tag="rs2", name=f"rs_{n}_{pair[0]}")
            nc.scalar.activation(rs, lnt, AF.Exp, scale=-0.5)
            yield
            for u, g in enumerate(pair):
                srcn = sp.tile([P, TC], BF16, tag="srcn", name=f"srcn_{n}_{g}")
                nc.vector.tensor_mul(srcn, stage[g], rs[:, u, :])
                yield
                qtr[g] = qsc.tile([P, TC], BF16, tag="qtr", name=f"qtr_{g}_{n}")
                _rope(nc, sp, srcn, qtr[g], 0, ropeC, ropeS, n, f"q{g}_{n}")
                yield

        stage = {}
        qtr = {}
        qtrs[n] = qtr
        yield from mwave([MK])
        yield from kchain()
        for tb in range(4):
            vn = _proj_vnat(nc, cps, wt, xns, n, tb)
            nc.vector.tensor_copy(Vnat[:, 4 * n + tb, :], vn[:, 0:KC])
            yield
        for pair in ([0, 1], [2, 3]):
            yield from mwave(pair)
            yield from qchains(pair)

    def attention(i, qtr, fill, norm_carry):
        """Causal attention for q-chunk i, all 4 heads. Score blocks are
        QK'd and exp'd in fused pairs; es/acc in fp16 with the CSH shift.
        Each head's normalization tail (accf/rps/rec/yo) is deferred until
        after the next head's first pair so its Pool/PE/DVE chain never
        stalls the PE queue (norm_carry[0] holds the pending closure)."""
        npair_off = 2 * i  # off-diagonal block pairs (all full-width)
        for g in range(G):
            yps = cps.tile([P, QC], F32, tag="yps", bufs=2, name=f"yps_{g}_{i}")
            acc2 = asb2.tile([P, 2, QC], FP16, tag="acc", name=f"acc_{g}_{i}")

            def qk_pair(jp, cl, ch, name):
                sps2 = cps.tile([P, 2, QC], F32, tag="sps", bufs=2, name=name)
                for u in range(2):
                    nc.tensor.matmul(
                        sps2[:, u, cl:ch],
                        KTr[:, (2 * jp + u) * KC : (2 * jp + u + 1) * KC],
                        qtr[g][:, cl:ch],
                        start=True,
                        stop=True,
                    )
                return sps2

            def exp_pair(jp, sps2, cl, ch, first):
                es2 = acc2 if first else asb.tile(
                    [P, 2, QC], FP16, tag="es", name=f"es_{g}_{i}_{jp}"
                )
                nc.scalar.activation(
                    es2[:, :, cl:ch], sps2[:, :, cl:ch], AF.Exp, bias=mC[:, :]
                )
                return es2

            def pv_pair(jp, es2, cl, ch, start, stop):
                for u in range(2):
                    nc.tensor.matmul(
                        yps[:, cl:ch],
                        Vnat[:, 2 * jp + u, :],
                        es2[:, u, cl:ch],
                        start=start and u == 0,
                        stop=stop and u == 1,
                    )

            for jp in range(npair_off):
                sps2 = qk_pair(jp, 0, QC, f"sps_{g}_{i}_{jp}")
                fill()
                es2 = exp_pair(jp, sps2, 0, QC, jp == 0)
                if jp == 0 and norm_carry[0] is not None:
                    norm_carry[0]()
                    norm_carry[0] = None
                if jp > 0:
                    nc.vector.tensor_add(acc2, acc2, es2)
                pv_pair(jp, es2, 0, QC, jp == 0, False)
                fill()

            # diag pair A: blocks 4i, 4i+1 (computed full-width; masks kill
            # the upper triangle on es)
            jpa = npair_off
            sps2 = qk_pair(jpa, 0, QC, f"spsA_{g}_{i}")
            fill()
            esA = exp_pair(jpa, sps2, 0, QC, npair_off == 0)
            if norm_carry[0] is not None:
                norm_carry[0]()
                norm_carry[0] = None
            nc.vector.tensor_mul(esA[:, 0, 0:KC], esA[:, 0, 0:KC], tri01)
            nc.vector.tensor_mul(esA[:, 1, 0 : 2 * KC], esA[:, 1, 0 : 2 * KC], mask2)
            if npair_off > 0:
                nc.vector.tensor_add(acc2, acc2, esA)
            pv_pair(jpa, esA, 0, QC, npair_off == 0, False)
            fill()

            # diag pair B: blocks 4i+2, 4i+3 (cols 256:512 only)
            jpb = npair_off + 1
            sps2 = qk_pair(jpb, 2 * KC, QC, f"spsB_{g}_{i}")
            fill()
            esB = asb.tile([P, 2, QC], FP16, tag="es", name=f"esB_{g}_{i}")
            nc.scalar.activation(
                esB[:, :, 2 * KC : QC], sps2[:, :, 2 * KC : QC], AF.Exp, bias=mC[:, :]
            )
            nc.vector.tensor_mul(
                esB[:, 0, 2 * KC : 3 * KC], esB[:, 0, 2 * KC : 3 * KC], tri01
            )
            nc.vector.tensor_mul(
                esB[:, 1, 2 * KC : QC], esB[:, 1, 2 * KC : QC], mask2
            )
            nc.vector.tensor_add(
                acc2[:, :, 2 * KC : QC], acc2[:, :, 2 * KC : QC], esB[:, :, 2 * KC : QC]
            )
            pv_pair(jpb, esB, 2 * KC, QC, False, True)
            fill()

            # normalization: accf = acc2[0] + acc2[1] on the (idle) gpsimd,
            # then one fp16 ones-matmul; y = yps * (1/rps). Emitted deferred
            # (see norm_carry) so the accf latency hides under the next head.
            accf = asb2.tile([P, QC], FP16, tag="accf", name=f"accf_{g}_{i}")
            nc.vector.tensor_add(accf, acc2[:, 0, :], acc2[:, 1, :])

            def norm_tail(g=g, yps=yps, accf=accf):
                rps = cps.tile([P, TC], F32, tag="pj", bufs=2, name=f"rps_{g}_{i}")
                nc.tensor.matmul(rps, ones_h, accf, start=True, stop=True)
                rec = asb2.tile([P, QC], F32, tag="rec", name=f"rec_{g}_{i}")
                nc.vector.reciprocal_approx_fast(out=rec, in_=rps)
                yo = asb.tile([P, QC], BF16, tag="yo", name=f"yo_{g}_{i}")
                nc.vector.tensor_mul(yo, yps, rec)
                nc.sync.dma_start(
                    out=YT[g * DH : (g + 1) * DH, i * QC : (i + 1) * QC], in_=yo
                )

            norm_carry[0] = norm_tail

    # ---- software pipeline: window_stream(n+1) is drained as PE-filler
    # inside attention(n); in-order engine queues then interleave ready
    # projection work into attention's exp-wait gaps ----
    qtrs = {}
    norm_carry = [None]
    for _ in window_stream(0, qtrs):
        pass
    for n in range(NT):
        stream = window_stream(n + 1, qtrs) if n + 1 < NT else iter(())

        def fill(k=2, _s=stream):
            for _ in range(k):
                next(_s, None)

        attention(n, qtrs.pop(n), fill, norm_carry)
        for _ in stream:
            pass
    norm_carry[0]()


def _pin_act_table_set():
    """Restrict the ACT table chooser to natural_log_exp_and_others (holds
    ln/exp/copy — every function this kernel uses) so the compiled stream
    has one table load instead of one per ln<->exp switch (~1.3us each)."""
    import concourse.hw_specs as hw_specs

    if getattr(bacc, "_act_tables_pinned", False):
        return
    orig = hw_specs.get_activation_tables
    keep = "natural_log_exp_and_others"

    def patched(arch):
        t = orig(arch)
        return {k: (v if k == keep else set()) for k, v in t.items()}

    bacc.get_activation_tables = patched
    bacc._act_tables_pinned = True


def build_nc(reps=1):
    _pin_act_table_set()
    nc = bacc.Bacc(trn_type="TRN2")
    nc.cur_io = {
        "xT": nc.dram_tensor("xT", [DCH, P, NT, TC], BF16, kind="ExternalInput"),
        "wT": nc.dram_tensor("wT", [P, DCH, ETOT], BF16, kind="ExternalInput"),
        "ropeC": nc.dram_tensor("ropeC", [P // 2, T], BF16, kind="ExternalInput"),
        "ropeS": nc.dram_tensor("ropeS", [P // 2, T], BF16, kind="ExternalInput"),
        "mask2": nc.dram_tensor("mask2", [P, 2 * KC], FP16, kind="ExternalInput"),
        "yT": nc.dram_tensor("yT", [EQ, T], BF16, kind="ExternalOutput"),
    }
    with tile.TileContext(nc) as tc:
        for _rep in range(reps):
            with ExitStack() as ctx:
                _body(nc, tc, ctx)
    nc.finalize()
    return nc


_NC_CACHE = None


def _get_nc():
    global _NC_CACHE
    if _NC_CACHE is None:
        _NC_CACHE = build_nc()
    return _NC_CACHE


def _host_tables():
    inv_freq = 1.0 / (ROPE_BASE ** (np.arange(0, DH, 2, dtype=np.float32) / DH))
    t = np.arange(T, dtype=np.float32)
    freqs = np.outer(t, inv_freq).astype(np.float32)     # (T, 64)
    ropeC = np.ascontiguousarray(np.cos(freqs).T).astype(ml_dtypes.bfloat16)
    ropeS = np.ascontiguousarray(np.sin(freqs).T).astype(ml_dtypes.bfloat16)
    pp_ = np.arange(KC)[:, None]
    ff = np.arange(KC)[None, :]
    tri01 = (pp_ <= ff).astype(np.float16)
    mask2 = np.concatenate([np.zeros((KC, KC), dtype=np.float16), tri01], axis=1)
    return ropeC, ropeS, mask2


def host_inmaps(x, Wq, Wk, Wv):
    x = np.asarray(x, dtype=np.float32)
    Wq = np.asarray(Wq, dtype=np.float32)
    Wk = np.asarray(Wk, dtype=np.float32)
    Wv = np.asarray(Wv, dtype=np.float32)
    ropeC, ropeS, mask2 = _host_tables()
    in_maps = []

    for core in range(NCORES):
        b, h = divmod(core, HKV)
        # xTb[d, p, n, t] = x[b, n*TC+t, d*128+p]
        xT = np.ascontiguousarray(x[b].T)                # (D, T)
        xTb = np.ascontiguousarray(
            xT.reshape(DCH, P, NT, TC)
        ).astype(ml_dtypes.bfloat16)
        Wsl = np.concatenate(
            [
                Wq[h * EQ : (h + 1) * EQ],
                Wk[h * DH : (h + 1) * DH],
                Wv[h * DH : (h + 1) * DH],
            ],
            axis=0,
        )                                                # (768, D)
        # wTb[p, d, e] = Wsl[e, d*128+p]
        wTb = np.ascontiguousarray(
            Wsl.T.reshape(DCH, P, ETOT).transpose(1, 0, 2)
        ).astype(ml_dtypes.bfloat16)
        in_maps.append(
            {
                "xT": xTb,
                "wT": wTb,
                "ropeC": ropeC,
                "ropeS": ropeS,
                "mask2": mask2,
            }
        )

    return in_maps


def kernel(x, Wq, Wk, Wv):
    in_maps = host_inmaps(x, Wq, Wk, Wv)
    nc = _get_nc()
    res = run_bass_kernel_spmd(nc, in_maps, core_ids=list(range(NCORES)))

    out = np.empty((B, T, H * DH), dtype=np.float32)
    for core in range(NCORES):
        b, h = divmod(core, HKV)
        yT = np.asarray(res.results[core]["yT"]).astype(np.float32)
        out[b, :, h * EQ : (h + 1) * EQ] = (
            yT.reshape(G, DH, T).transpose(2, 0, 1).reshape(T, EQ)
        )
    return out


# revision 7
# speedup vs baseline: 1.0514x; 1.0514x over previous
"""MixerAttention (GQA + QK-RMSNorm + RoPE + causal) Trainium2 kernel, v2.

Sharding: 8 cores = batch(2) x kv-head(4). Fully local per core — no collectives.

v2 redesign (v1 was DVE-bound at ~94% busy, ACT 72%, PE 67%):
  - Projections run in bf16 (fp8 was measured at 3.4e-2 max-normalized error
    vs the 2e-2 gate -- peaked softmax amplifies score noise -- while the
    all-bf16/fp16 pipeline lands at 4e-3).
  - V is projected directly in natural (token, DH) orientation by swapping
    stationary/moving operands, eliminating the PE transposes and their
    staging copies.
  - All DVE chain math is 16-bit (bf16/fp16) in SBUF, hitting the DVE 2x/4x
    packed modes; only PSUM-touching ops stay 4-byte.
  - Softmax runs shifted: es = exp(s - 9) <= e^2.5 so es/acc live in fp16;
    the shift cancels in the normalization. Causal masking is a fp16
    multiply on es (mask01), not a -inf add on PSUM scores.
  - Score blocks are exp'd in fused pairs ([128, 2, 512] PSUM tiles) to
    halve ACT instruction count; denominators accumulate on the DVE in fp16
    and finish with one fp16 ones-matmul per (head, q-chunk).
  - Projections for window n+1 are emitted before attention(i=n) so the PE
    has dense work while attention waits on ACT exps.
"""
import sys

sys.path.insert(0, "/opt/trn_rl_repo")
from contextlib import ExitStack

import ml_dtypes
import numpy as np
import concourse.bacc as bacc
import concourse.mybir as mybir
import concourse.tile as tile
from concourse.bass_utils import run_bass_kernel_spmd

F32 = mybir.dt.float32
BF16 = mybir.dt.bfloat16
FP16 = mybir.dt.float16
FP8 = mybir.dt.float8e4
AF = mybir.ActivationFunctionType
DR = mybir.MatmulPerfMode.DoubleRow

B, T, D = 2, 2048, 2048
H, HKV, DH = 16, 4, 128
G = H // HKV                    # q heads per kv head (per core)
EPS = 1.1920928955078125e-07
ROPE_BASE = 10000.0
NCORES = 8

CSH = 9.0                       # softmax shift: es = exp(s - CSH), s in [-11.4, 11.4]

P = 128                         # partitions
DCH = D // P                    # 16 contraction chunks
NT = 4                          # column windows of 512
TC = T // NT                    # 512
EQ = G * DH                     # 512
ETOT = EQ + DH + DH             # 768
QC = 512                        # attention q-chunk == TC
KC = 128                        # attention k-chunk
NKC = T // KC                   # 16
MK, MV = G, G + 1               # m-tile indices of k and v rows
BQ = float(P) * EPS             # rms eps bias, q (1/sqrt(DH) folded via scale=1)
BK = EPS                        # rms eps bias, k


def _proj_wave(nc, pools, wt, xns, n, ms):
    """One projection wave for window n: bf16-accumulate the m-tiles in
    `ms` ([P,TC] out each). Returns dict m -> psum tile."""
    cps = pools
    psl = {m: cps.tile([P, TC], F32, tag="pj", bufs=2, name=f"pj_{n}_{m}") for m in ms}
    for d in range(DCH):
        xn = xns[(n, d)]
        for m in ms:
            nc.tensor.matmul(
                psl[m],
                wt[:, d, m * P : (m + 1) * P],
                xn,
                start=d == 0,
                stop=d == DCH - 1,
            )
    return psl


def _proj_vnat(nc, cps, wt, xns, n, tb):
    """Natural-orientation V projection for token block tb of window n:
    out [P tokens, KC dh] — one sequential accumulation group per psum bank
    (PSUM allows a single live group per 2KB zero region)."""
    vn = cps.tile([P, TC], F32, tag="pj", bufs=2, name=f"vn_{n}_{tb}")
    for d in range(DCH):
        xn = xns[(n, d)]
        nc.tensor.matmul(
            vn[:, 0:KC],
            xn[:, tb * KC : (tb + 1) * KC],
            wt[:, d, MV * P : (MV + 1) * P],
            start=d == 0,
            stop=d == DCH - 1,
        )
    return vn


def _rope(nc, sp, srcn, dst, dst0, ropeC, ropeS, n, label):
    """dst[:, dst0:dst0+TC] = srcn*C + rot_half(srcn)*S, all bf16 on DVE.
    The rotate-half uses partition-crossed copies (walrus allows those but
    not partition-crossed tensor_tensor inputs); ropeS holds [s; -s]."""
    c0 = n * TC
    h = P // 2
    tmp = sp.tile([P, TC], BF16, tag="rtmp", name=f"rtmp_{label}")
    nc.vector.tensor_copy(tmp[0:h, :], srcn[h:P, :])
    nc.vector.tensor_copy(tmp[h:P, :], srcn[0:h, :])
    t1 = sp.tile([P, TC], BF16, tag="rt1", name=f"rt1_{label}")
    nc.vector.tensor_mul(t1, srcn, ropeC[:, c0 : c0 + TC])
    nc.vector.tensor_mul(tmp, tmp, ropeS[:, c0 : c0 + TC])
    nc.vector.tensor_add(dst[:, dst0 : dst0 + TC], t1, tmp)


def _make_pools(tc, ctx):
    """Pools are created ONCE and shared across reps so the tile rings
    rotate across rep boundaries: rep r+1's weights/KTr/Vnat land in the
    other buffer instead of WAR-serializing behind rep r's last readers."""
    return {
        "constp": ctx.enter_context(tc.tile_pool(name="const", bufs=2)),
        "finp": ctx.enter_context(tc.tile_pool(name="final", bufs=2)),
        "wp": ctx.enter_context(tc.tile_pool(name="wp", bufs=2)),
        "xp": ctx.enter_context(tc.tile_pool(name="xp", bufs=17)),
        "xp2": ctx.enter_context(tc.tile_pool(name="xp2", bufs=17)),
        "stg": ctx.enter_context(tc.tile_pool(name="stg", bufs=10)),
        "sp": ctx.enter_context(tc.tile_pool(name="sp", bufs=2)),
        "qsc": ctx.enter_context(tc.tile_pool(name="qsc", bufs=10)),
        "asb": ctx.enter_context(tc.tile_pool(name="asb", bufs=6)),
        "asb2": ctx.enter_context(tc.tile_pool(name="asb2", bufs=2)),
        "cps": ctx.enter_context(tc.tile_pool(name="cps", bufs=1, space="PSUM")),
    }


def _body(nc, tc, pools, rep):
    XT = nc.cur_io["xT"]
    WT = nc.cur_io["wT"]
    RC = nc.cur_io["ropeC"]
    RS_ = nc.cur_io["ropeS"]
    MSK = nc.cur_io["mask2"]
    YT = nc.cur_io["yT"]

    constp = pools["constp"]
    finp = pools["finp"]
    wp = pools["wp"]
    xp = pools["xp"]
    xp2 = pools["xp2"]
    stg = pools["stg"]
    sp = pools["sp"]
    qsc = pools["qsc"]
    asb = pools["asb"]
    asb2 = pools["asb2"]
    cps = pools["cps"]

    # weights split per d-chunk pair and interleaved with window-0 x loads so
    # the first projection matmuls unblock fast; x in [P, TC] bf16 tiles
    wt = wp.tile([P, DCH, ETOT], BF16, tag="wt")
    xns = {}

    def load_x(n, d):
        xn = xp.tile([P, TC], BF16, tag="xn", name=f"xn_{n}_{d}")
        nc.sync.dma_start(out=xn, in_=XT[d, :, n, :])
        xns[(n, d)] = xn

    def load_x23(d):
        xn = xp2.tile([P, 2, TC], BF16, tag="xn2", name=f"xn23_{d}")
        nc.sync.dma_start(out=xn, in_=XT[d, :, 2:4, :])
        xns[(2, d)] = xn[:, 0, :]
        xns[(3, d)] = xn[:, 1, :]

    for d2 in range(DCH // 2):
        nc.sync.dma_start(
            out=wt[:, 2 * d2 : 2 * d2 + 2, :], in_=WT[:, 2 * d2 : 2 * d2 + 2, :]
        )
        load_x(0, 2 * d2)
        load_x(0, 2 * d2 + 1)

    mask2 = constp.tile([P, 2 * KC], FP16, tag="mask2")
    nc.sync.dma_start(out=mask2, in_=MSK[:, :])
    tri01 = mask2[:, KC : 2 * KC]
    # rope halves are duplicated ([c;c], [s;-s]) — upload 64 rows, fill the
    # second partition half on-chip
    ropeC = constp.tile([P, T], BF16, tag="ropeC")
    nc.sync.dma_start(out=ropeC[0 : P // 2, :], in_=RC[:, :])
    nc.vector.tensor_copy(ropeC[P // 2 : P, :], ropeC[0 : P // 2, :])
    ropeS = constp.tile([P, T], BF16, tag="ropeS")
    nc.sync.dma_start(out=ropeS[0 : P // 2, :], in_=RS_[:, :])
    nc.vector.tensor_scalar_mul(ropeS[P // 2 : P, :], ropeS[0 : P // 2, :], -1.0)
    ones_bf = constp.tile([P, P], BF16, tag="ones_bf")
    nc.vector.memset(ones_bf, 1.0)
    ones_h = constp.tile([P, P], FP16, tag="ones_h")
    nc.vector.memset(ones_h, 1.0)
    bq = constp.tile([P, 1], F32, tag="bq")
    nc.vector.memset(bq, BQ)
    bk = constp.tile([P, 1], F32, tag="bk")
    nc.vector.memset(bk, BK)
    mC = constp.tile([P, 1], F32, tag="mC")
    nc.vector.memset(mC, -CSH)

    KTr = finp.tile([P, T], BF16, tag="KTr")
    Vnat = finp.tile([P, NKC, KC], FP16, tag="Vnat")

    def stage_of(psl, n, m):
        st = stg.tile([P, TC], BF16, tag="stage", name=f"st_{n}_{m}")
        nc.vector.tensor_copy(st, psl[m])
        return st

    def window_stream(n, qtrs):
        """Generator emitting projections + chains for window n in small
        batches (yield points). Drained as PE-filler inside attention(n-1),
        so the in-order PE queue interleaves ready projection matmuls with
        exp-gated attention matmuls. Also prefetches x for window n+1.
        Stores the window's qtr dict into qtrs[n]."""
        if n == 1:
            for d in range(0, DCH, 4):
                for dd in range(d, d + 4):
                    load_x23(dd)
                yield
        elif n == 0:
            for d in range(0, DCH, 4):
                for dd in range(d, d + 4):
                    load_x(1, dd)
                yield

        def mwave(ms):
            psl = {
                m: cps.tile([P, TC], F32, tag="pj", bufs=2, name=f"pj_{n}_{m}")
                for m in ms
            }
            for d in range(DCH):
                for m in ms:
                    nc.tensor.matmul(
                        psl[m],
                        wt[:, d, m * P : (m + 1) * P],
                        xns[(n, d)],
                        start=d == 0,
                        stop=d == DCH - 1,
                    )
                if len(ms) * (d + 1) % 4 == 0:
                    yield
            for m in ms:
                stage[m] = stage_of(psl, n, m)
            yield

        def kchain():
            sqk = sp.tile([P, TC], BF16, tag="sq", name=f"sqk_{n}")
            nc.vector.tensor_mul(sqk, stage[MK], stage[MK])
            ssbk = cps.tile([P, 2, QC], F32, tag="sps", bufs=2, name=f"ssbk_{n}")
            nc.tensor.matmul(ssbk[:, 0, :], ones_bf, sqk, start=True, stop=True)
            yield
            lntk = sp.tile([P, TC], F32, tag="lnt", name=f"lntk_{n}")
            nc.scalar.activation(
                lntk, ssbk[:, 0, :], AF.Ln, scale=1.0 / P, bias=bk[:, :]
            )
            rsk = sp.tile([P, TC], FP16, tag="rs", name=f"rsk_{n}")
            nc.scalar.activation(rsk, lntk, AF.Exp, scale=-0.5)
            yield
            srcnk = sp.tile([P, TC], BF16, tag="srcn", name=f"srcnk_{n}")
            nc.vector.tensor_mul(srcnk, stage[MK], rsk)
            yield
            _rope(nc, sp, srcnk, KTr, n * TC, ropeC, ropeS, n, f"k{n}")
            yield

        def qchains(pair):
            sqs = {}
            ssb = cps.tile(
                [P, 2, QC], F32, tag="sps", bufs=2, name=f"ssb_{n}_{pair[0]}"
            )
            for u, g in enumerate(pair):
                sqs[g] = sp.tile([P, TC], BF16, tag="sq", name=f"sq_{n}_{g}")
                nc.vector.tensor_mul(sqs[g], stage[g], stage[g])
                nc.tensor.matmul(ssb[:, u, :], ones_bf, sqs[g], start=True, stop=True)
                yield
            lnt = sp.tile([P, 2, TC], F32, tag="lnt2", name=f"lnt_{n}_{pair[0]}")
            nc.scalar.activation(lnt, ssb, AF.Ln, scale=1.0, bias=bq[:, :])
            rs = sp.tile([P, 2, TC], FP16, tag="rs2", name=f"rs_{n}_{pair[0]}")
            nc.scalar.activation(rs, lnt, AF.Exp, scale=-0.5)
            yield
            for u, g in enumerate(pair):
                srcn = sp.tile([P, TC], BF16, tag="srcn", name=f"srcn_{n}_{g}")
                nc.vector.tensor_mul(srcn, stage[g], rs[:, u, :])
                yield
                qtr[g] = qsc.tile([P, TC], BF16, tag="qtr", name=f"qtr_{g}_{n}")
                _rope(nc, sp, srcn, qtr[g], 0, ropeC, ropeS, n, f"q{g}_{n}")
                yield

        stage = {}
        qtr = {}
        qtrs[n] = qtr
        yield from mwave([MK])
        yield from kchain()
        for tb in range(4):
            vn = _proj_vnat(nc, cps, wt, xns, n, tb)
            nc.vector.tensor_copy(Vnat[:, 4 * n + tb, :], vn[:, 0:KC])
            yield
        for pair in ([0, 1], [2, 3]):
            yield from mwave(pair)
            yield from qchains(pair)

    def attention(i, qtr, fill, norm_carry):
        """Causal attention for q-chunk i, all 4 heads. Score blocks are
        QK'd and exp'd in fused pairs; es/acc in fp16 with the CSH shift.
        Each head's normalization tail (accf/rps/rec/yo) is deferred until
        after the next head's first pair so its Pool/PE/DVE chain never
        stalls the PE queue (norm_carry[0] holds the pending closure)."""
        npair_off = 2 * i  # off-diagonal block pairs (all full-width)
        for g in range(G):
            yps = cps.tile([P, QC], F32, tag="yps", bufs=2, name=f"yps_{g}_{i}")
            acc2 = asb2.tile([P, 2, QC], FP16, tag="acc", name=f"acc_{g}_{i}")

            def qk_pair(jp, cl, ch, name):
                sps2 = cps.tile([P, 2, QC], F32, tag="sps", bufs=2, name=name)
                for u in range(2):
                    nc.tensor.matmul(
                        sps2[:, u, cl:ch],
                        KTr[:, (2 * jp + u) * KC : (2 * jp + u + 1) * KC],
                        qtr[g][:, cl:ch],
                        start=True,
                        stop=True,
                    )
                return sps2

            def exp_pair(jp, sps2, cl, ch, first):
                es2 = acc2 if first else asb.tile(
                    [P, 2, QC], FP16, tag="es", name=f"es_{g}_{i}_{jp}"
                )
                nc.scalar.activation(
                    es2[:, :, cl:ch], sps2[:, :, cl:ch], AF.Exp, bias=mC[:, :]
                )
                return es2

            def pv_pair(jp, es2, cl, ch, start, stop):
                for u in range(2):
                    nc.tensor.matmul(
                        yps[:, cl:ch],
                        Vnat[:, 2 * jp + u, :],
                        es2[:, u, cl:ch],
                        start=start and u == 0,
                        stop=stop and u == 1,
                    )

            for jp in range(npair_off):
                sps2 = qk_pair(jp, 0, QC, f"sps_{g}_{i}_{jp}")
                fill()
                es2 = exp_pair(jp, sps2, 0, QC, jp == 0)
                if jp == 0 and norm_carry[0] is not None:
                    norm_carry[0]()
                    norm_carry[0] = None
                if jp > 0:
                    nc.vector.tensor_add(acc2, acc2, es2)
                pv_pair(jp, es2, 0, QC, jp == 0, False)
                fill()

            # diag pair A: blocks 4i, 4i+1 (computed full-width; masks kill
            # the upper triangle on es)
            jpa = npair_off
            sps2 = qk_pair(jpa, 0, QC, f"spsA_{g}_{i}")
            fill()
            esA = exp_pair(jpa, sps2, 0, QC, npair_off == 0)
            if norm_carry[0] is not None:
                norm_carry[0]()
                norm_carry[0] = None
            nc.vector.tensor_mul(esA[:, 0, 0:KC], esA[:, 0, 0:KC], tri01)
            nc.vector.tensor_mul(esA[:, 1, 0 : 2 * KC], esA[:, 1, 0 : 2 * KC], mask2)
            if npair_off > 0:
                nc.vector.tensor_add(acc2, acc2, esA)
            pv_pair(jpa, esA, 0, QC, npair_off == 0, False)
            fill()

            # diag pair B: blocks 4i+2, 4i+3 (cols 256:512 only)
            jpb = npair_off + 1
            sps2 = qk_pair(jpb, 2 * KC, QC, f"spsB_{g}_{i}")
            fill()
            esB = asb.tile([P, 2, QC], FP16, tag="es", name=f"esB_{g}_{i}")
            nc.scalar.activation(
                esB[:, :, 2 * KC : QC], sps2[:, :, 2 * KC : QC], AF.Exp, bias=mC[:, :]
            )
            nc.vector.tensor_mul(
                esB[:, 0, 2 * KC : 3 * KC], esB[:, 0, 2 * KC : 3 * KC], tri01
            )
            nc.vector.tensor_mul(
                esB[:, 1, 2 * KC : QC], esB[:, 1, 2 * KC : QC], mask2
            )
            nc.vector.tensor_add(
                acc2[:, :, 2 * KC : QC], acc2[:, :, 2 * KC : QC], esB[:, :, 2 * KC : QC]
            )
            pv_pair(jpb, esB, 2 * KC, QC, False, True)
            fill()

            # normalization: accf = acc2[0] + acc2[1] on the (idle) gpsimd,
            # then one fp16 ones-matmul; y = yps * (1/rps). Emitted deferred
            # (see norm_carry) so the accf latency hides under the next head.
            accf = asb2.tile([P, QC], FP16, tag="accf", name=f"accf_{g}_{i}")
            nc.vector.tensor_add(accf, acc2[:, 0, :], acc2[:, 1, :])

            def norm_tail(g=g, yps=yps, accf=accf):
                rps = cps.tile([P, TC], F32, tag="pj", bufs=2, name=f"rps_{g}_{i}")
                nc.tensor.matmul(rps, ones_h, accf, start=True, stop=True)
                rec = asb2.tile([P, QC], F32, tag="rec", name=f"rec_{g}_{i}")
                nc.vector.reciprocal_approx_fast(out=rec, in_=rps)
                yo = asb.tile([P, QC], BF16, tag="yo", name=f"yo_{g}_{i}")
                nc.vector.tensor_mul(yo, yps, rec)
                nc.sync.dma_start(
                    out=YT[g * DH : (g + 1) * DH, i * QC : (i + 1) * QC], in_=yo
                )

            norm_carry[0] = norm_tail

    # ---- software pipeline: window_stream(n+1) is drained as PE-filler
    # inside attention(n); in-order engine queues then interleave ready
    # projection work into attention's exp-wait gaps ----
    qtrs = {}
    norm_carry = [None]
    for _ in window_stream(0, qtrs):
        pass
    for n in range(NT):
        stream = window_stream(n + 1, qtrs) if n + 1 < NT else iter(())

        def fill(k=2, _s=stream):
            for _ in range(k):
                next(_s, None)

        attention(n, qtrs.pop(n), fill, norm_carry)
        for _ in stream:
            pass
    norm_carry[0]()


def _pin_act_table_set():
    """Restrict the ACT table chooser to natural_log_exp_and_others (holds
    ln/exp/copy — every function this kernel uses) so the compiled stream
    has one table load instead of one per ln<->exp switch (~1.3us each)."""
    import concourse.hw_specs as hw_specs

    if getattr(bacc, "_act_tables_pinned", False):
        return
    orig = hw_specs.get_activation_tables
    keep = "natural_log_exp_and_others"

    def patched(arch):
        t = orig(arch)
        return {k: (v if k == keep else set()) for k, v in t.items()}

    bacc.get_activation_tables = patched
    bacc._act_tables_pinned = True


def build_nc(reps=1):
    _pin_act_table_set()
    nc = bacc.Bacc(trn_type="TRN2")
    nc.cur_io = {
        "xT": nc.dram_tensor("xT", [DCH, P, NT, TC], BF16, kind="ExternalInput"),
        "wT": nc.dram_tensor("wT", [P, DCH, ETOT], BF16, kind="ExternalInput"),
        "ropeC": nc.dram_tensor("ropeC", [P // 2, T], BF16, kind="ExternalInput"),
        "ropeS": nc.dram_tensor("ropeS", [P // 2, T], BF16, kind="ExternalInput"),
        "mask2": nc.dram_tensor("mask2", [P, 2 * KC], FP16, kind="ExternalInput"),
        "yT": nc.dram_tensor("yT", [EQ, T], BF16, kind="ExternalOutput"),
    }
    with tile.TileContext(nc) as tc:
        with ExitStack() as ctx:
            pools = _make_pools(tc, ctx)
            for _rep in range(reps):
                _body(nc, tc, pools, _rep)
    nc.finalize()
    return nc


_NC_CACHE = None


def _get_nc():
    global _NC_CACHE
    if _NC_CACHE is None:
        _NC_CACHE = build_nc()
    return _NC_CACHE


def _host_tables():
    inv_freq = 1.0 / (ROPE_BASE ** (np.arange(0, DH, 2, dtype=np.float32) / DH))
    t = np.arange(T, dtype=np.float32)
    freqs = np.outer(t, inv_freq).astype(np.float32)     # (T, 64)
    ropeC = np.ascontiguousarray(np.cos(freqs).T).astype(ml_dtypes.bfloat16)
    ropeS = np.ascontiguousarray(np.sin(freqs).T).astype(ml_dtypes.bfloat16)
    pp_ = np.arange(KC)[:, None]
    ff = np.arange(KC)[None, :]
    tri01 = (pp_ <= ff).astype(np.float16)
    mask2 = np.concatenate([np.zeros((KC, KC), dtype=np.float16), tri01], axis=1)
    return ropeC, ropeS, mask2


def host_inmaps(x, Wq, Wk, Wv):
    x = np.asarray(x, dtype=np.float32)
    Wq = np.asarray(Wq, dtype=np.float32)
    Wk = np.asarray(Wk, dtype=np.float32)
    Wv = np.asarray(Wv, dtype=np.float32)
    ropeC, ropeS, mask2 = _host_tables()
    in_maps = []

    for core in range(NCORES):
        b, h = divmod(core, HKV)
        # xTb[d, p, n, t] = x[b, n*TC+t, d*128+p]
        xT = np.ascontiguousarray(x[b].T)                # (D, T)
        xTb = np.ascontiguousarray(
            xT.reshape(DCH, P, NT, TC)
        ).astype(ml_dtypes.bfloat16)
        Wsl = np.concatenate(
            [
                Wq[h * EQ : (h + 1) * EQ],
                Wk[h * DH : (h + 1) * DH],
                Wv[h * DH : (h + 1) * DH],
            ],
            axis=0,
        )                                                # (768, D)
        # wTb[p, d, e] = Wsl[e, d*128+p]
        wTb = np.ascontiguousarray(
            Wsl.T.reshape(DCH, P, ETOT).transpose(1, 0, 2)
        ).astype(ml_dtypes.bfloat16)
        in_maps.append(
            {
                "xT": xTb,
                "wT": wTb,
                "ropeC": ropeC,
                "ropeS": ropeS,
                "mask2": mask2,
            }
        )

    return in_maps


def kernel(x, Wq, Wk, Wv):
    in_maps = host_inmaps(x, Wq, Wk, Wv)
    nc = _get_nc()
    res = run_bass_kernel_spmd(nc, in_maps, core_ids=list(range(NCORES)))

    out = np.empty((B, T, H * DH), dtype=np.float32)
    for core in range(NCORES):
        b, h = divmod(core, HKV)
        yT = np.asarray(res.results[core]["yT"]).astype(np.float32)
        out[b, :, h * EQ : (h + 1) * EQ] = (
            yT.reshape(G, DH, T).transpose(2, 0, 1).reshape(T, EQ)
        )
    return out
